# revision 31
# baseline (speedup 1.0000x reference)
"""Multi-head attention (B=4, T=2048, D=2048, H=16) on 8 Trainium2 cores.

Sharding v4 (head-parallel pairs + pairwise AllGather, pipelined):
  core c -> batch b = c//2, role r = c%2. Core handles 8 heads
  (heads r*8..r*8+7) over the FULL sequence of its batch.

  Pipeline: phase A projects V(ev0) + Q/K for head 0 only. Then 8 head
  "slots": slot h runs attention for head h while the PE also computes
  Q/K for head h+1 (and V ev1 during slots 0-3) as filler work. This
  keeps the PE dense while the scalar engine's exp stream (the real
  co-bottleneck) runs concurrently. Q never round-trips through DRAM.

  Softmax denominator: instead of a full 128-row ones-matmul per key
  chunk (1/7 of all PE cycles in v3), exp outputs are pair-summed and
  chain-accumulated on DVE ([128,512] tiles, sum over the 16 key
  chunks), then ONE small ones-matmul per (head, tq) reduces over
  partitions and broadcasts. Reciprocal via reciprocal_approx_fast.

  exp is issued on [128,2,512] PSUM pairs (two score banks per
  activation) to halve ACT instruction overhead. K/V PSUM evacuations
  run on the scalar engine (Copy) to keep DVE headroom.

  Exchange: pairwise AllGather of the partner-row half of attn^T,
  fired as soon as the needed heads' partner rows are done (mid slot 5
  and mid slot 7). Received blocks are mask-selected (per-core 0/1
  masks) on gpsimd/DVE during the out_proj phase.

All matmuls run in bf16. DRAM inputs are pre-laid-out host-side in
SBUF tile order; x is chunked so the first matmul starts after ~2.5MB
of DMA instead of 8MB.
"""
import sys
if '/opt/trn_rl_repo' not in sys.path:
    sys.path.insert(0, '/opt/trn_rl_repo')

import math
import numpy as np

import concourse.bass as bass
import concourse.mybir as mybir
import concourse.tile as tile
from concourse import bacc

F32 = mybir.dt.float32
BF16 = mybir.dt.bfloat16

D = 2048          # model dim
DH = 128          # head dim
DC = D // 128     # d-dim chunks of x (16)
HL = 8            # heads per core
EL = HL * DH      # local e-dims (1024)
SCALE = 1.0 / math.sqrt(DH)

PAIRS = [[0, 1], [2, 3], [4, 5], [6, 7]]
XBLOCKS = ((0, 6), (6, 2))   # exchange blocks: (first head, n heads)

Copy = mybir.ActivationFunctionType.Copy
Exp = mybir.ActivationFunctionType.Exp


def build_body(nc, tc, ctx, aps, T):
    TH = T // 2                # my token half
    KC = T // 128              # key chunks
    TT = T // 512              # 512-wide token tiles
    NKG = KC // 2              # key-chunk pairs per tq block
    xt, wq, wk, wv, wo, bq, bo, ones, msel, y = (
        aps['xt'], aps['wq'], aps['wk'], aps['wv'], aps['wo'],
        aps['bq'], aps['bo'], aps['ones'], aps['msel'], aps['y'])

    singles = ctx.enter_context(tc.tile_pool(name='singles', bufs=1))
    dram = ctx.enter_context(tc.tile_pool(name='dram', bufs=1, space='DRAM'))
    stream = ctx.enter_context(tc.tile_pool(name='wstream', bufs=2))
    attn_pool = ctx.enter_context(tc.tile_pool(name='attn', bufs=1))

    bq_sb = singles.tile([128, HL], F32)
    nc.scalar.dma_start(out=bq_sb, in_=bq.rearrange('c p -> p c'))
    ones_sb = singles.tile([128, 128], BF16)
    nc.scalar.dma_start(out=ones_sb, in_=ones)
    msel_sb = singles.tile([128, 2], F32)
    nc.scalar.dma_start(out=msel_sb, in_=msel)
    # trigger the exp table-set DMA (~2.7us) under phase A
    scratch = singles.tile([128, 2], F32)
    nc.scalar.activation(scratch, msel_sb, Exp)

    sends = [dram.tile([128, n, TH], BF16, name=f'send{i}')
             for i, (_, n) in enumerate(XBLOCKS)]
    recvs = [dram.tile([2, 128, n, TH], BF16, name=f'recv{i}')
             for i, (_, n) in enumerate(XBLOCKS)]

    attn_sb = attn_pool.tile([128, HL, T], BF16)   # attn_out^T (my heads)

    def exchange(blk):
        """AllGather partner-row half of attn^T for head block blk."""
        lo, n = XBLOCKS[blk]
        nc.sync.dma_start(
            out=sends[blk], in_=attn_sb[:, lo:lo + n, TH:T])
        nc.gpsimd.collective_compute(
            'AllGather', mybir.AluOpType.bypass, replica_groups=PAIRS,
            ins=[sends[blk][:]], outs=[recvs[blk][:]])

    with tc.tile_pool(name='kv', bufs=1) as kvp, \
         tc.tile_pool(name='kq', bufs=2) as kqp, \
         tc.tile_pool(name='epool', bufs=3) as epool, \
         tc.tile_pool(name='cpool', bufs=3) as cpool, \
         tc.tile_pool(name='ripool', bufs=2) as ripool, \
         tc.tile_pool(name='ps_s', bufs=2, space='PSUM') as ps_s, \
         tc.tile_pool(name='ps_o', bufs=2, space='PSUM') as ps_o, \
         tc.tile_pool(name='ps_f', bufs=2, space='PSUM') as ps_f:

        v_sb = kvp.tile([128, KC, EL], BF16)       # V  [key%128, kc, dim]
        qts, kts = {}, {}

        def attention_block(h, tq, pop=None):
            # pop(k): emit k pending filler micro-ops (single matmuls of
            # the next head's projections). Interleaved per key-pair
            # because the exp stream (1147ns/pair) is slower than the
            # block's own PE work (853ns/pair) and Tile's cost model
            # doesn't know that (it omits ACT's 352-cycle overhead).
            qsl = slice(tq * 512, (tq + 1) * 512)
            hsl = slice(h * 128, (h + 1) * 128)
            last = (h == HL - 1)
            o2 = ps_o.tile([128, 512], F32, tag='o2')
            state = {'S': None}
            sps, es = {}, {}

            def emit_scores(j):
                sp = ps_s.tile([128, 2, 512], F32, tag='s', name='sp')
                sps[j] = sp
                for half in range(2):
                    kc = 2 * j + half
                    nc.tensor.matmul(
                        sp[:, half, :], kts[h][:, kc * 128:(kc + 1) * 128],
                        qts[h][:, qsl], start=True, stop=True)

            def emit_exp(j):
                e = epool.tile([128, 2, 512], BF16, tag='e', name='e')
                es[j] = e
                nc.scalar.activation(e, sps[j], Exp, scale=SCALE)

            def emit_av(j):
                e = es[j]
                for half in range(2):
                    kc = 2 * j + half
                    nc.tensor.matmul(
                        o2, v_sb[:, kc, hsl], e[:, half, :],
                        start=(kc == 0), stop=(kc == KC - 1))
                t = cpool.tile([128, 512], BF16, tag='ct')
                # last slot: DVE is its bottleneck — offload half the
                # pair-adds to the (otherwise idle) gpsimd engine
                eng = nc.gpsimd if (last and j % 2) else nc.vector
                eng.tensor_add(t, e[:, 0, :], e[:, 1, :])
                S = state['S']
                if S is None:
                    state['S'] = t
                else:
                    S2 = cpool.tile([128, 512], BF16, tag='cs', bufs=2)
                    nc.vector.tensor_add(S2, S, t)
                    state['S'] = S2

            # software pipeline: the attnV consumers trail the scores
            # producers by two pairs, so exp's real latency (which the
            # scheduler's cost model understates) is already paid when
            # the attnV matmuls reach the head of the PE queue
            emit_scores(0)
            emit_scores(1)
            emit_exp(0)
            for j in range(NKG):
                if j + 2 < NKG:
                    emit_scores(j + 2)
                if j + 1 < NKG:
                    emit_exp(j + 1)
                emit_av(j)
                if pop is not None:
                    pop(2)
            S = state['S']
            if last:
                # the filler psum ring is idle in the last slot; using it
                # for sm decouples this block's denominator tail from the
                # next block's attnV start (o2 ring pressure)
                sm = ps_f.tile([128, 512], F32, tag='fill', name='sm7')
            else:
                sm = ps_o.tile([128, 512], F32, tag='o2')
            nc.tensor.matmul(sm, ones_sb, S, start=True, stop=True)
            ri = ripool.tile([128, 512], F32, tag='ri')
            nc.vector.reciprocal_approx_fast(ri, sm)
            nc.vector.tensor_mul(attn_sb[:, h, qsl], o2, ri)

        with tc.tile_pool(name='xt', bufs=1) as xtp, \
             tc.tile_pool(name='wqk', bufs=2) as wqkp:
            xt_sb = xtp.tile([128, DC, T], BF16)
            for c in range(TT):
                nc.sync.dma_start(
                    out=xt_sb[:, :, c * 512:(c + 1) * 512], in_=xt[c])
            wv_tiles = [stream.tile([128, DC, 512], BF16, tag='ws',
                                    name=f'wv{ev}') for ev in range(2)]

            def new_head(g):
                wq_sb = wqkp.tile([128, DC, 128], BF16, tag='wq')
                nc.scalar.dma_start(out=wq_sb, in_=wq[g])
                wk_sb = wqkp.tile([128, DC, 128], BF16, tag='wk')
                nc.scalar.dma_start(out=wk_sb, in_=wk[g])
                qts[g] = kqp.tile([128, T], BF16, tag='qt', name=f'qt{g}')
                kts[g] = kqp.tile([128, T], BF16, tag='kt', name=f'kt{g}')
                return wq_sb, wk_sb

            def proj_qk_group(which, g, tt, w_sb):
                ps = ps_f.tile([128, 512], F32, tag='fill')
                tsl = slice(tt * 512, (tt + 1) * 512)
                for d in range(DC):
                    nc.tensor.matmul(
                        ps, w_sb[:, d, :], xt_sb[:, d, tsl],
                        start=(d == 0), stop=(d == DC - 1))
                if which == 'q':
                    nc.vector.tensor_scalar_add(
                        qts[g][:, tsl], ps, bq_sb[:, g:g + 1])
                else:
                    nc.scalar.activation(kts[g][:, tsl], ps, Copy)

            def proj_v_group(ev, ti):
                ps = ps_f.tile([128, 512], F32, tag='fill')
                for d in range(DC):
                    nc.tensor.matmul(
                        ps, xt_sb[:, d, ti * 128:(ti + 1) * 128],
                        wv_tiles[ev][:, d, :],
                        start=(d == 0), stop=(d == DC - 1))
                nc.scalar.activation(
                    v_sb[:, ti, ev * 512:(ev + 1) * 512], ps, Copy)

            # ---------------- phase A ----------------
            # weight DMA order on the scalar ring: wq0/wk0 first (the
            # first matmuls need them), then the big wv tiles
            wq0, wk0 = new_head(0)
            nc.scalar.dma_start(out=wv_tiles[0], in_=wv[0])
            nc.scalar.dma_start(out=wv_tiles[1], in_=wv[1])
            vpt = KC // TT             # V ti-groups per tt chunk
            for tt in range(TT):
                proj_qk_group('q', 0, tt, wq0)
                proj_qk_group('k', 0, tt, wk0)
                for ti in range(vpt * tt, vpt * tt + vpt):
                    proj_v_group(0, ti)

            # ---------------- head slots ----------------
            # filler projections are queued as single-matmul micro-ops so
            # they can be woven INTO the attention blocks (see
            # attention_block's pop comment)
            def qk_micro(which, g, tt, w_sb):
                st = {}
                tsl = slice(tt * 512, (tt + 1) * 512)

                def mk(d):
                    def f():
                        if d == 0:
                            st['ps'] = ps_f.tile([128, 512], F32,
                                                 tag='fill', name='fps')
                        nc.tensor.matmul(
                            st['ps'], w_sb[:, d, :], xt_sb[:, d, tsl],
                            start=(d == 0), stop=(d == DC - 1))
                    return f

                def fin():
                    if which == 'q':
                        nc.vector.tensor_scalar_add(
                            qts[g][:, tsl], st['ps'], bq_sb[:, g:g + 1])
                    else:
                        nc.scalar.activation(kts[g][:, tsl], st['ps'], Copy)
                return [mk(d) for d in range(DC)] + [fin]

            def v_micro(ev, ti):
                st = {}

                def mk(d):
                    def f():
                        if d == 0:
                            st['ps'] = ps_f.tile([128, 512], F32,
                                                 tag='fill', name='fps')
                        nc.tensor.matmul(
                            st['ps'], xt_sb[:, d, ti * 128:(ti + 1) * 128],
                            wv_tiles[ev][:, d, :],
                            start=(d == 0), stop=(d == DC - 1))
                    return f

                def fin():
                    nc.scalar.activation(
                        v_sb[:, ti, ev * 512:(ev + 1) * 512], st['ps'], Copy)
                return [mk(d) for d in range(DC)] + [fin]

            nvs = (KC + 3) // 4        # V groups per slot (slots 0..3)
            tq_order = list(range(TT // 2, TT)) + list(range(TT // 2))
            wo_tiles = []
            for h in range(HL):
                if h == 2:
                    # out_proj weights stream into the slots wv vacates;
                    # issue on the (idle) sync ring, not behind the exps
                    for ne in range(4):
                        wo_tiles.append(stream.tile(
                            [128, DC, 512], BF16, tag='ws', name=f'wo{ne}'))
                    nc.sync.dma_start(out=wo_tiles[0], in_=wo[0])
                    nc.sync.dma_start(out=wo_tiles[1], in_=wo[1])
                g = h + 1
                micro = []
                if g < HL:
                    wqg, wkg = new_head(g)
                    for tt in range(TT):
                        micro += qk_micro('q', g, tt, wqg)
                        micro += qk_micro('k', g, tt, wkg)
                if h < 4:
                    for ti in range(nvs * h, min(nvs * h + nvs, KC)):
                        micro += v_micro(1, ti)
                mstate = {'i': 0}

                def pop(k, micro=micro, mstate=mstate):
                    j = mstate['i']
                    for fn in micro[j:j + k]:
                        fn()
                    mstate['i'] = min(j + k, len(micro))

                nblocks = len(tq_order)
                for idx, tq in enumerate(tq_order):
                    attention_block(h, tq, pop=pop if micro else None)
                    if idx == TT // 2 - 1:
                        if h == 5:
                            exchange(0)
                        if h == HL - 1:
                            exchange(1)
                    # even out the remainder across the rest of the slot
                    remaining = len(micro) - mstate['i']
                    if remaining > 0:
                        pop(remaining // (nblocks - idx) if idx < nblocks - 1
                            else remaining)

    # ---------------- out_proj ----------------
    with tc.tile_pool(name='tail', bufs=1) as tailp, \
         tc.tile_pool(name='psum3', bufs=1, space='PSUM') as psum3:
        bo_sb = tailp.tile([128, D], BF16)
        nc.sync.dma_start(out=bo_sb, in_=bo)
        # gather -> SBUF, select partner block with per-core masks:
        # t0 = r2[0]*msel0 on ACT (Copy with scale), then one fused
        # DVE op rem = r2[1]*msel1 + t0
        rem_sb = tailp.tile([128, HL, TH], BF16)
        for blk, (lo, n) in enumerate(XBLOCKS):
            rr = recvs[blk].rearrange('j p c t -> p j c t')
            for c0 in range(0, n, 2):
                r2 = tailp.tile([128, 2, 2, TH], BF16, tag='r2', bufs=2)
                nc.sync.dma_start(out=r2, in_=rr[:, :, c0:c0 + 2, :])
                t0 = tailp.tile([128, 2, TH], BF16, tag='t0', bufs=2)
                nc.scalar.activation(t0, r2[:, 0], Copy,
                                     scale=msel_sb[:, 0:1])
                nc.vector.scalar_tensor_tensor(
                    rem_sb[:, lo + c0:lo + c0 + 2, :], r2[:, 1],
                    msel_sb[:, 1:2], t0,
                    op0=mybir.AluOpType.mult, op1=mybir.AluOpType.add)

        n_ti = TH // 128
        c_first = list(range(7)) + list(range(8, 14))
        for ne in range(4):
            if ne >= 2:
                nc.sync.dma_start(out=wo_tiles[ne], in_=wo[ne])
            wo_sb = wo_tiles[ne]
            esl = slice(ne * 512, (ne + 1) * 512)
            # accumulate the chunks that are ready earliest (heads 0-6 +
            # first exchange block) for the whole group, holding PSUM, so
            # head 7 and the last exchange block have maximal slack
            pss = []
            for ti in range(n_ti):
                ps = psum3.tile([128, 512], F32, tag=f'ps{ti}')
                tsl = slice(ti * 128, (ti + 1) * 128)
                for i, c in enumerate(c_first):
                    lhsT = (attn_sb[:, c, tsl] if c < 8
                            else rem_sb[:, c - 8, tsl])
                    nc.tensor.matmul(
                        ps, lhsT, wo_sb[:, c, :],
                        start=(i == 0), stop=False)
                pss.append(ps)
            for ti in range(n_ti):
                ps = pss[ti]
                tsl = slice(ti * 128, (ti + 1) * 128)
                for c in (7, 14, 15):
                    lhsT = (attn_sb[:, c, tsl] if c < 8
                            else rem_sb[:, c - 8, tsl])
                    nc.tensor.matmul(
                        ps, lhsT, wo_sb[:, c, :],
                        start=False, stop=(c == 15))
                o = tailp.tile([128, 512], F32, tag='y', bufs=3)
                nc.vector.tensor_add(o, ps, bo_sb[:, esl])
                nc.sync.dma_start(out=y[tsl, esl], in_=o)


def build_nc(T=2048, reps=1):
    import contextlib
    nc = bacc.Bacc('TRN2', target_bir_lowering=False, debug=False)
    TH = T // 2
    TT = T // 512
    t = {}
    t['xt'] = nc.dram_tensor('xt', [TT, 128, DC, 512], BF16,
                             kind='ExternalInput')
    for w in ('wq', 'wk'):
        t[w] = nc.dram_tensor(w, [8, 128, DC, 128], BF16,
                              kind='ExternalInput')
    t['wv'] = nc.dram_tensor('wv', [2, 128, DC, 512], BF16,
                             kind='ExternalInput')
    t['wo'] = nc.dram_tensor('wo', [4, 128, DC, 512], BF16,
                             kind='ExternalInput')
    t['bq'] = nc.dram_tensor('bq', [HL, 128], F32, kind='ExternalInput')
    t['bo'] = nc.dram_tensor('bo', [128, D], BF16, kind='ExternalInput')
    t['ones'] = nc.dram_tensor('ones', [128, 128], BF16, kind='ExternalInput')
    t['msel'] = nc.dram_tensor('msel', [128, 2], F32, kind='ExternalInput')
    t['y'] = nc.dram_tensor('y', [TH, D], F32, kind='ExternalOutput')
    aps = {k: v.ap() for k, v in t.items()}
    with tile.TileContext(nc) as tc:
        with contextlib.ExitStack() as ctx:
            if reps > 1:
                with tc.For_i(0, reps, 1):
                    with contextlib.ExitStack() as ctx2:
                        build_body(nc, tc, ctx2, aps, T)
            else:
                build_body(nc, tc, ctx, aps, T)
    nc.compile()
    return nc


def _bf16(a):
    import ml_dtypes
    return np.asarray(a, dtype=ml_dtypes.bfloat16)


def _sbuf_layout(w, width):
    """[D, n*width] -> [n, 128, DC, width] matching SBUF tile order."""
    n = w.shape[1] // width
    blocks = []
    for i in range(n):
        b = w[:, i * width:(i + 1) * width]
        blocks.append(b.reshape(DC, 128, width).transpose(1, 0, 2))
    return np.ascontiguousarray(np.stack(blocks))


def make_inputs(x, qkv_w, qkv_b, out_w, out_b):
    """Host-side shard/layout prep. Returns list of 8 per-core input dicts."""
    B, T, _ = x.shape
    TH = T // 2
    TT = T // 512
    wq_t = np.ascontiguousarray(qkv_w[0:D].T)          # [D, D] in->out
    wk_t = np.ascontiguousarray(qkv_w[D:2 * D].T)
    wv_t = np.ascontiguousarray(qkv_w[2 * D:3 * D].T)
    wo_t = np.ascontiguousarray(out_w.T)               # [d_in, e_out]
    bo_vec = out_b + out_w @ qkv_b[2 * D:3 * D]
    bo = _bf16(np.broadcast_to(bo_vec, (128, D)))
    ones = _bf16(np.ones((128, 128), np.float32))
    xts = [np.ascontiguousarray(x[b].T) for b in range(B)]
    ins = []
    for c in range(8):
        b, r = c // 2, c % 2
        el = slice(r * EL, (r + 1) * EL)
        rem_el = slice((1 - r) * EL, (2 - r) * EL)
        # rotated token order: own half first
        xbt = xts[b]
        xrot = np.concatenate(
            [xbt[:, r * TH:(r + 1) * TH], xbt[:, (1 - r) * TH:(2 - r) * TH]],
            axis=1)
        xrot = _bf16(xrot)
        xt4 = np.stack([
            xrot[:, cc * 512:(cc + 1) * 512].reshape(DC, 128, 512)
            .transpose(1, 0, 2) for cc in range(TT)])
        # out_proj weights: local-head rows then partner-head rows
        wo_cat = np.concatenate([wo_t[el, :], wo_t[rem_el, :]], axis=0)
        msel = np.zeros((128, 2), np.float32)
        msel[:, 1 - r] = 1.0   # pick partner block (even picks 1, odd 0)
        ins.append({
            'xt': np.ascontiguousarray(xt4),
            'wq': _sbuf_layout(_bf16(wq_t[:, el]), 128),
            'wk': _sbuf_layout(_bf16(wk_t[:, el]), 128),
            'wv': _sbuf_layout(_bf16(wv_t[:, el]), 512),
            'wo': _sbuf_layout(_bf16(wo_cat), 512),
            'bq': np.ascontiguousarray(
                qkv_b[r * EL:(r + 1) * EL].reshape(HL, 128)).astype(
                    np.float32),
            'bo': bo,
            'ones': ones,
            'msel': msel,
        })
    return ins


class SpmdRunner:
    """SPMD runner over axon PJRT keeping a reusable jitted callable."""

    def __init__(self, nc, n_cores=8):
        import jax
        from jax.sharding import Mesh, PartitionSpec
        from jax.experimental.shard_map import shard_map
        from concourse import bass2jax
        bass2jax.install_neuronx_cc_hook()
        self.nc = nc
        self.n_cores = n_cores
        partition_name = (
            nc.partition_id_tensor.name if nc.partition_id_tensor else None)
        in_names, out_names, out_avals, zero_outs = [], [], [], []
        for alloc in nc.m.functions[0].allocations:
            if not isinstance(alloc, mybir.MemoryLocationSet):
                continue
            name = alloc.memorylocations[0].name
            if alloc.kind == 'ExternalInput':
                if name != partition_name:
                    in_names.append(name)
            elif alloc.kind == 'ExternalOutput':
                shape = tuple(alloc.tensor_shape)
                dtype = mybir.dt.np(alloc.dtype)
                out_names.append(name)
                out_avals.append(jax.core.ShapedArray(shape, dtype))
                zero_outs.append(np.zeros(shape, dtype))
        self.in_names = in_names
        self.out_names = out_names
        self.out_avals = out_avals
        self.zero_outs = zero_outs
        self.n_params = len(in_names)
        n_outs = len(out_avals)
        all_in_names = list(in_names) + list(out_names)
        if partition_name is not None:
            all_in_names.append(partition_name)

        def _body(*args):
            operands = list(args)
            if partition_name is not None:
                operands.append(bass2jax.partition_id_tensor())
            outs = bass2jax._bass_exec_p.bind(
                *operands,
                out_avals=tuple(out_avals),
                in_names=tuple(all_in_names),
                out_names=tuple(out_names),
                lowering_input_output_aliases=(),
                sim_require_finite=True,
                sim_require_nnan=True,
                nc=nc,
            )
            return tuple(outs)

        import os
        if os.environ.get('BASS_SIM'):
            devices = jax.devices('cpu')[:n_cores]
        else:
            devices = jax.devices()[:n_cores]
        assert len(devices) == n_cores
        self.mesh = Mesh(np.asarray(devices), ('core',))
        in_specs = (PartitionSpec('core'),) * (self.n_params + n_outs)
        out_specs = (PartitionSpec('core'),) * n_outs
        self.fn = jax.jit(
            shard_map(_body, mesh=self.mesh, in_specs=in_specs,
                      out_specs=out_specs, check_rep=False),
            keep_unused=True)
        self._jax = jax

    def pack(self, in_maps):
        per_core = [[np.asarray(m[n]) for n in self.in_names] for m in in_maps]
        concat_in = [
            np.concatenate([per_core[c][i] for c in range(self.n_cores)],
                           axis=0)
            for i in range(self.n_params)]
        concat_zeros = [
            np.zeros((self.n_cores * z.shape[0], *z.shape[1:]), z.dtype)
            for z in self.zero_outs]
        return concat_in + concat_zeros

    def device_put(self, args):
        from jax.sharding import NamedSharding, PartitionSpec
        sh = NamedSharding(self.mesh, PartitionSpec('core'))
        return [self._jax.device_put(a, sh) for a in args]

    def unpack(self, out_arrs):
        return [
            {n: np.asarray(out_arrs[i]).reshape(
                self.n_cores, *self.out_avals[i].shape)[c]
             for i, n in enumerate(self.out_names)}
            for c in range(self.n_cores)]

    def run(self, in_maps):
        return self.unpack(self.fn(*self.pack(in_maps)))

    def time_exec(self, in_maps, iters=20, warmup=3):
        import time as _time
        args = self.device_put(self.pack(in_maps))
        out = None
        for _ in range(warmup):
            out = self.fn(*args)
        self._jax.block_until_ready(out)
        t0 = _time.perf_counter()
        outs = [self.fn(*args) for _ in range(iters)]
        self._jax.block_until_ready(outs)
        return (_time.perf_counter() - t0) / iters


_CACHE = {}


def _get_runner(T=2048, reps=1):
    key = (T, reps)
    if key not in _CACHE:
        nc = build_nc(T=T, reps=reps)
        _CACHE[key] = SpmdRunner(nc, 8)
    return _CACHE[key]


def kernel(x, qkv_w, qkv_b, out_w, out_b):
    B, T, _ = x.shape
    TH = T // 2
    runner = _get_runner(T=T)
    ins = make_inputs(x, qkv_w, qkv_b, out_w, out_b)
    res = runner.run(ins)
    out = np.empty((B, T, D), np.float32)
    for c in range(8):
        b, r = c // 2, c % 2
        out[b, r * TH:(r + 1) * TH, :] = res[c]['y']
    return out


# revision 34
# speedup vs baseline: 1.0076x; 1.0076x over previous
"""Multi-head attention (B=4, T=2048, D=2048, H=16) on 8 Trainium2 cores.

Sharding v4 (head-parallel pairs + pairwise AllGather, pipelined):
  core c -> batch b = c//2, role r = c%2. Core handles 8 heads
  (heads r*8..r*8+7) over the FULL sequence of its batch.

  Pipeline: phase A projects V(ev0) + Q/K for head 0 only. Then 8 head
  "slots": slot h runs attention for head h while the PE also computes
  Q/K for head h+1 (and V ev1 during slots 0-3) as filler work. This
  keeps the PE dense while the scalar engine's exp stream (the real
  co-bottleneck) runs concurrently. Q never round-trips through DRAM.

  Softmax denominator: instead of a full 128-row ones-matmul per key
  chunk (1/7 of all PE cycles in v3), exp outputs are pair-summed and
  chain-accumulated on DVE ([128,512] tiles, sum over the 16 key
  chunks), then ONE small ones-matmul per (head, tq) reduces over
  partitions and broadcasts. Reciprocal via reciprocal_approx_fast.

  exp is issued on [128,2,512] PSUM pairs (two score banks per
  activation) to halve ACT instruction overhead. K/V PSUM evacuations
  run on the scalar engine (Copy) to keep DVE headroom.

  Exchange: pairwise AllGather of the partner-row half of attn^T,
  fired as soon as the needed heads' partner rows are done (mid slot 5
  and mid slot 7). Received blocks are mask-selected (per-core 0/1
  masks) on gpsimd/DVE during the out_proj phase.

All matmuls run in bf16. DRAM inputs are pre-laid-out host-side in
SBUF tile order; x is chunked so the first matmul starts after ~2.5MB
of DMA instead of 8MB.
"""
import sys
if '/opt/trn_rl_repo' not in sys.path:
    sys.path.insert(0, '/opt/trn_rl_repo')

import math
import numpy as np

import concourse.bass as bass
import concourse.mybir as mybir
import concourse.tile as tile
from concourse import bacc

F32 = mybir.dt.float32
BF16 = mybir.dt.bfloat16

D = 2048          # model dim
DH = 128          # head dim
DC = D // 128     # d-dim chunks of x (16)
HL = 8            # heads per core
EL = HL * DH      # local e-dims (1024)
SCALE = 1.0 / math.sqrt(DH)

PAIRS = [[0, 1], [2, 3], [4, 5], [6, 7]]
XBLOCKS = ((0, 6), (6, 2))   # exchange blocks: (first head, n heads)

Copy = mybir.ActivationFunctionType.Copy
Exp = mybir.ActivationFunctionType.Exp


def build_body(nc, tc, ctx, aps, T):
    TH = T // 2                # my token half
    KC = T // 128              # key chunks
    TT = T // 512              # 512-wide token tiles
    NKG = KC // 2              # key-chunk pairs per tq block
    xt, wq, wk, wv, wo, bq, bo, ones, msel, y = (
        aps['xt'], aps['wq'], aps['wk'], aps['wv'], aps['wo'],
        aps['bq'], aps['bo'], aps['ones'], aps['msel'], aps['y'])

    singles = ctx.enter_context(tc.tile_pool(name='singles', bufs=1))
    dram = ctx.enter_context(tc.tile_pool(name='dram', bufs=1, space='DRAM'))
    stream = ctx.enter_context(tc.tile_pool(name='wstream', bufs=2))
    attn_pool = ctx.enter_context(tc.tile_pool(name='attn', bufs=1))

    bq_sb = singles.tile([128, HL], F32)
    nc.scalar.dma_start(out=bq_sb, in_=bq.rearrange('c p -> p c'))
    ones_sb = singles.tile([128, 128], BF16)
    nc.scalar.dma_start(out=ones_sb, in_=ones)
    msel_sb = singles.tile([128, 2], F32)
    nc.scalar.dma_start(out=msel_sb, in_=msel)
    # trigger the exp table-set DMA (~2.7us) under phase A
    scratch = singles.tile([128, 2], F32)
    nc.scalar.activation(scratch, msel_sb, Exp)

    sends = [dram.tile([128, n, TH], BF16, name=f'send{i}')
             for i, (_, n) in enumerate(XBLOCKS)]
    recvs = [dram.tile([2, 128, n, TH], BF16, name=f'recv{i}')
             for i, (_, n) in enumerate(XBLOCKS)]

    attn_sb = attn_pool.tile([128, HL, T], BF16)   # attn_out^T (my heads)

    def exchange(blk):
        """AllGather partner-row half of attn^T for head block blk."""
        lo, n = XBLOCKS[blk]
        nc.sync.dma_start(
            out=sends[blk], in_=attn_sb[:, lo:lo + n, TH:T])
        nc.gpsimd.collective_compute(
            'AllGather', mybir.AluOpType.bypass, replica_groups=PAIRS,
            ins=[sends[blk][:]], outs=[recvs[blk][:]])

    with tc.tile_pool(name='kv', bufs=1) as kvp, \
         tc.tile_pool(name='kq', bufs=2) as kqp, \
         tc.tile_pool(name='epool', bufs=3) as epool, \
         tc.tile_pool(name='cpool', bufs=3) as cpool, \
         tc.tile_pool(name='ripool', bufs=2) as ripool, \
         tc.tile_pool(name='ps_s', bufs=2, space='PSUM') as ps_s, \
         tc.tile_pool(name='ps_o', bufs=2, space='PSUM') as ps_o, \
         tc.tile_pool(name='ps_f', bufs=2, space='PSUM') as ps_f:

        v_sb = kvp.tile([128, KC, EL], BF16)       # V  [key%128, kc, dim]
        qts, kts = {}, {}

        def attention_block(h, tq, pop=None):
            # pop(k): emit k pending filler micro-ops (single matmuls of
            # the next head's projections). Interleaved per key-pair
            # because the exp stream (1147ns/pair) is slower than the
            # block's own PE work (853ns/pair) and Tile's cost model
            # doesn't know that (it omits ACT's 352-cycle overhead).
            qsl = slice(tq * 512, (tq + 1) * 512)
            hsl = slice(h * 128, (h + 1) * 128)
            last = (h == HL - 1)
            o2 = ps_o.tile([128, 512], F32, tag='o2')
            state = {'S': None}
            sps, es = {}, {}

            def emit_scores(j):
                sp = ps_s.tile([128, 2, 512], F32, tag='s', name='sp')
                sps[j] = sp
                for half in range(2):
                    kc = 2 * j + half
                    nc.tensor.matmul(
                        sp[:, half, :], kts[h][:, kc * 128:(kc + 1) * 128],
                        qts[h][:, qsl], start=True, stop=True)

            def emit_exp(j):
                e = epool.tile([128, 2, 512], BF16, tag='e', name='e')
                es[j] = e
                nc.scalar.activation(e, sps[j], Exp, scale=SCALE)

            def emit_av(j):
                e = es[j]
                for half in range(2):
                    kc = 2 * j + half
                    nc.tensor.matmul(
                        o2, v_sb[:, kc, hsl], e[:, half, :],
                        start=(kc == 0), stop=(kc == KC - 1))
                t = cpool.tile([128, 512], BF16, tag='ct')
                # last slot: DVE is its bottleneck — offload half the
                # pair-adds to the (otherwise idle) gpsimd engine
                eng = nc.gpsimd if (last and j % 2) else nc.vector
                eng.tensor_add(t, e[:, 0, :], e[:, 1, :])
                S = state['S']
                if S is None:
                    state['S'] = t
                else:
                    S2 = cpool.tile([128, 512], BF16, tag='cs', bufs=2)
                    nc.vector.tensor_add(S2, S, t)
                    state['S'] = S2

            # software pipeline: the attnV consumers trail the scores
            # producers by two pairs, so exp's real latency (which the
            # scheduler's cost model understates) is already paid when
            # the attnV matmuls reach the head of the PE queue
            emit_scores(0)
            emit_scores(1)
            emit_exp(0)
            for j in range(NKG):
                if j + 2 < NKG:
                    emit_scores(j + 2)
                if j + 1 < NKG:
                    emit_exp(j + 1)
                emit_av(j)
                if pop is not None:
                    pop(2)
            S = state['S']
            sm = ps_o.tile([128, 512], F32, tag='o2')
            nc.tensor.matmul(sm, ones_sb, S, start=True, stop=True)
            ri = ripool.tile([128, 512], F32, tag='ri')
            nc.vector.reciprocal_approx_fast(ri, sm)
            nc.vector.tensor_mul(attn_sb[:, h, qsl], o2, ri)

        with tc.tile_pool(name='xt', bufs=1) as xtp, \
             tc.tile_pool(name='wqk', bufs=2) as wqkp:
            xt_sb = xtp.tile([128, DC, T], BF16)
            for c in range(TT):
                nc.sync.dma_start(
                    out=xt_sb[:, :, c * 512:(c + 1) * 512], in_=xt[c])
            wv_tiles = [stream.tile([128, DC, 512], BF16, tag='ws',
                                    name=f'wv{ev}') for ev in range(2)]

            def new_head(g):
                wq_sb = wqkp.tile([128, DC, 128], BF16, tag='wq')
                nc.scalar.dma_start(out=wq_sb, in_=wq[g])
                wk_sb = wqkp.tile([128, DC, 128], BF16, tag='wk')
                nc.scalar.dma_start(out=wk_sb, in_=wk[g])
                qts[g] = kqp.tile([128, T], BF16, tag='qt', name=f'qt{g}')
                kts[g] = kqp.tile([128, T], BF16, tag='kt', name=f'kt{g}')
                return wq_sb, wk_sb

            def proj_qk_group(which, g, tt, w_sb):
                ps = ps_f.tile([128, 512], F32, tag='fill')
                tsl = slice(tt * 512, (tt + 1) * 512)
                for d in range(DC):
                    nc.tensor.matmul(
                        ps, w_sb[:, d, :], xt_sb[:, d, tsl],
                        start=(d == 0), stop=(d == DC - 1))
                if which == 'q':
                    nc.vector.tensor_scalar_add(
                        qts[g][:, tsl], ps, bq_sb[:, g:g + 1])
                else:
                    nc.scalar.activation(kts[g][:, tsl], ps, Copy)

            def proj_v_group(ev, ti):
                ps = ps_f.tile([128, 512], F32, tag='fill')
                for d in range(DC):
                    nc.tensor.matmul(
                        ps, xt_sb[:, d, ti * 128:(ti + 1) * 128],
                        wv_tiles[ev][:, d, :],
                        start=(d == 0), stop=(d == DC - 1))
                nc.scalar.activation(
                    v_sb[:, ti, ev * 512:(ev + 1) * 512], ps, Copy)

            # ---------------- phase A ----------------
            # weight DMA order on the scalar ring: wq0/wk0 first (the
            # first matmuls need them), then the big wv tiles
            wq0, wk0 = new_head(0)
            nc.scalar.dma_start(out=wv_tiles[0], in_=wv[0])
            nc.scalar.dma_start(out=wv_tiles[1], in_=wv[1])
            vpt = KC // TT             # V ti-groups per tt chunk
            for tt in range(TT):
                proj_qk_group('q', 0, tt, wq0)
                proj_qk_group('k', 0, tt, wk0)
                for ti in range(vpt * tt, vpt * tt + vpt):
                    proj_v_group(0, ti)

            # ---------------- head slots ----------------
            # filler projections are queued as single-matmul micro-ops so
            # they can be woven INTO the attention blocks (see
            # attention_block's pop comment)
            def qk_micro(which, g, tt, w_sb):
                st = {}
                tsl = slice(tt * 512, (tt + 1) * 512)

                def mk(d):
                    def f():
                        if d == 0:
                            st['ps'] = ps_f.tile([128, 512], F32,
                                                 tag='fill', name='fps')
                        nc.tensor.matmul(
                            st['ps'], w_sb[:, d, :], xt_sb[:, d, tsl],
                            start=(d == 0), stop=(d == DC - 1))
                    return f

                def fin():
                    if which == 'q':
                        nc.vector.tensor_scalar_add(
                            qts[g][:, tsl], st['ps'], bq_sb[:, g:g + 1])
                    else:
                        nc.scalar.activation(kts[g][:, tsl], st['ps'], Copy)
                return [mk(d) for d in range(DC)] + [fin]

            def v_micro(ev, ti):
                st = {}

                def mk(d):
                    def f():
                        if d == 0:
                            st['ps'] = ps_f.tile([128, 512], F32,
                                                 tag='fill', name='fps')
                        nc.tensor.matmul(
                            st['ps'], xt_sb[:, d, ti * 128:(ti + 1) * 128],
                            wv_tiles[ev][:, d, :],
                            start=(d == 0), stop=(d == DC - 1))
                    return f

                def fin():
                    nc.scalar.activation(
                        v_sb[:, ti, ev * 512:(ev + 1) * 512], st['ps'], Copy)
                return [mk(d) for d in range(DC)] + [fin]

            nvs = (KC + 3) // 4        # V groups per slot (slots 0..3)
            tq_order = list(range(TT // 2, TT)) + list(range(TT // 2))
            wo_tiles = []
            for h in range(HL):
                if h == 2:
                    # out_proj weights stream into the slots wv vacates;
                    # issue on the (idle) sync ring, not behind the exps
                    for ne in range(4):
                        wo_tiles.append(stream.tile(
                            [128, DC, 512], BF16, tag='ws', name=f'wo{ne}'))
                    nc.sync.dma_start(out=wo_tiles[0], in_=wo[0])
                    nc.sync.dma_start(out=wo_tiles[1], in_=wo[1])
                g = h + 1
                micro = []
                if g < HL:
                    wqg, wkg = new_head(g)
                    for tt in range(TT):
                        micro += qk_micro('q', g, tt, wqg)
                        micro += qk_micro('k', g, tt, wkg)
                if h < 4:
                    for ti in range(nvs * h, min(nvs * h + nvs, KC)):
                        micro += v_micro(1, ti)
                mstate = {'i': 0}

                def pop(k, micro=micro, mstate=mstate):
                    j = mstate['i']
                    for fn in micro[j:j + k]:
                        fn()
                    mstate['i'] = min(j + k, len(micro))

                nblocks = len(tq_order)
                for idx, tq in enumerate(tq_order):
                    attention_block(h, tq, pop=pop if micro else None)
                    if idx == TT // 2 - 1:
                        if h == 5:
                            exchange(0)
                        if h == HL - 1:
                            exchange(1)
                    # even out the remainder across the rest of the slot
                    remaining = len(micro) - mstate['i']
                    if remaining > 0:
                        pop(remaining // (nblocks - idx) if idx < nblocks - 1
                            else remaining)

    # ---------------- out_proj ----------------
    with tc.tile_pool(name='tail', bufs=1) as tailp, \
         tc.tile_pool(name='psum3', bufs=1, space='PSUM') as psum3:
        bo_sb = tailp.tile([128, D], BF16)
        nc.sync.dma_start(out=bo_sb, in_=bo)
        # gather -> SBUF, select partner block with per-core masks:
        # t0 = r2[0]*msel0 on ACT (Copy with scale), then one fused
        # DVE op rem = r2[1]*msel1 + t0
        rem_sb = tailp.tile([128, HL, TH], BF16)
        for blk, (lo, n) in enumerate(XBLOCKS):
            rr = recvs[blk].rearrange('j p c t -> p j c t')
            for c0 in range(0, n, 2):
                r2 = tailp.tile([128, 2, 2, TH], BF16, tag='r2', bufs=2)
                nc.sync.dma_start(out=r2, in_=rr[:, :, c0:c0 + 2, :])
                t0 = tailp.tile([128, 2, TH], BF16, tag='t0', bufs=2)
                nc.scalar.activation(t0, r2[:, 0], Copy,
                                     scale=msel_sb[:, 0:1])
                nc.vector.scalar_tensor_tensor(
                    rem_sb[:, lo + c0:lo + c0 + 2, :], r2[:, 1],
                    msel_sb[:, 1:2], t0,
                    op0=mybir.AluOpType.mult, op1=mybir.AluOpType.add)

        n_ti = TH // 128
        c_first = list(range(7)) + list(range(8, 14))
        for ne in range(4):
            if ne >= 2:
                nc.sync.dma_start(out=wo_tiles[ne], in_=wo[ne])
            wo_sb = wo_tiles[ne]
            esl = slice(ne * 512, (ne + 1) * 512)
            # accumulate the chunks that are ready earliest (heads 0-6 +
            # first exchange block) for the whole group, holding PSUM, so
            # head 7 and the last exchange block have maximal slack
            pss = []
            for ti in range(n_ti):
                ps = psum3.tile([128, 512], F32, tag=f'ps{ti}')
                tsl = slice(ti * 128, (ti + 1) * 128)
                for i, c in enumerate(c_first):
                    lhsT = (attn_sb[:, c, tsl] if c < 8
                            else rem_sb[:, c - 8, tsl])
                    nc.tensor.matmul(
                        ps, lhsT, wo_sb[:, c, :],
                        start=(i == 0), stop=False)
                pss.append(ps)
            for ti in range(n_ti):
                ps = pss[ti]
                tsl = slice(ti * 128, (ti + 1) * 128)
                for c in (7, 14, 15):
                    lhsT = (attn_sb[:, c, tsl] if c < 8
                            else rem_sb[:, c - 8, tsl])
                    nc.tensor.matmul(
                        ps, lhsT, wo_sb[:, c, :],
                        start=False, stop=(c == 15))
                o = tailp.tile([128, 512], F32, tag='y', bufs=3)
                nc.vector.tensor_add(o, ps, bo_sb[:, esl])
                nc.sync.dma_start(out=y[tsl, esl], in_=o)


def build_nc(T=2048, reps=1):
    import contextlib
    nc = bacc.Bacc('TRN2', target_bir_lowering=False, debug=False)
    TH = T // 2
    TT = T // 512
    t = {}
    t['xt'] = nc.dram_tensor('xt', [TT, 128, DC, 512], BF16,
                             kind='ExternalInput')
    for w in ('wq', 'wk'):
        t[w] = nc.dram_tensor(w, [8, 128, DC, 128], BF16,
                              kind='ExternalInput')
    t['wv'] = nc.dram_tensor('wv', [2, 128, DC, 512], BF16,
                             kind='ExternalInput')
    t['wo'] = nc.dram_tensor('wo', [4, 128, DC, 512], BF16,
                             kind='ExternalInput')
    t['bq'] = nc.dram_tensor('bq', [HL, 128], F32, kind='ExternalInput')
    t['bo'] = nc.dram_tensor('bo', [128, D], BF16, kind='ExternalInput')
    t['ones'] = nc.dram_tensor('ones', [128, 128], BF16, kind='ExternalInput')
    t['msel'] = nc.dram_tensor('msel', [128, 2], F32, kind='ExternalInput')
    t['y'] = nc.dram_tensor('y', [TH, D], F32, kind='ExternalOutput')
    aps = {k: v.ap() for k, v in t.items()}
    with tile.TileContext(nc) as tc:
        with contextlib.ExitStack() as ctx:
            if reps > 1:
                with tc.For_i(0, reps, 1):
                    with contextlib.ExitStack() as ctx2:
                        build_body(nc, tc, ctx2, aps, T)
            else:
                build_body(nc, tc, ctx, aps, T)
    nc.compile()
    return nc


def _bf16(a):
    import ml_dtypes
    return np.asarray(a, dtype=ml_dtypes.bfloat16)


def _sbuf_layout(w, width):
    """[D, n*width] -> [n, 128, DC, width] matching SBUF tile order."""
    n = w.shape[1] // width
    blocks = []
    for i in range(n):
        b = w[:, i * width:(i + 1) * width]
        blocks.append(b.reshape(DC, 128, width).transpose(1, 0, 2))
    return np.ascontiguousarray(np.stack(blocks))


def make_inputs(x, qkv_w, qkv_b, out_w, out_b):
    """Host-side shard/layout prep. Returns list of 8 per-core input dicts."""
    B, T, _ = x.shape
    TH = T // 2
    TT = T // 512
    wq_t = np.ascontiguousarray(qkv_w[0:D].T)          # [D, D] in->out
    wk_t = np.ascontiguousarray(qkv_w[D:2 * D].T)
    wv_t = np.ascontiguousarray(qkv_w[2 * D:3 * D].T)
    wo_t = np.ascontiguousarray(out_w.T)               # [d_in, e_out]
    bo_vec = out_b + out_w @ qkv_b[2 * D:3 * D]
    bo = _bf16(np.broadcast_to(bo_vec, (128, D)))
    ones = _bf16(np.ones((128, 128), np.float32))
    xts = [np.ascontiguousarray(x[b].T) for b in range(B)]
    ins = []
    for c in range(8):
        b, r = c // 2, c % 2
        el = slice(r * EL, (r + 1) * EL)
        rem_el = slice((1 - r) * EL, (2 - r) * EL)
        # rotated token order: own half first
        xbt = xts[b]
        xrot = np.concatenate(
            [xbt[:, r * TH:(r + 1) * TH], xbt[:, (1 - r) * TH:(2 - r) * TH]],
            axis=1)
        xrot = _bf16(xrot)
        xt4 = np.stack([
            xrot[:, cc * 512:(cc + 1) * 512].reshape(DC, 128, 512)
            .transpose(1, 0, 2) for cc in range(TT)])
        # out_proj weights: local-head rows then partner-head rows
        wo_cat = np.concatenate([wo_t[el, :], wo_t[rem_el, :]], axis=0)
        msel = np.zeros((128, 2), np.float32)
        msel[:, 1 - r] = 1.0   # pick partner block (even picks 1, odd 0)
        ins.append({
            'xt': np.ascontiguousarray(xt4),
            'wq': _sbuf_layout(_bf16(wq_t[:, el]), 128),
            'wk': _sbuf_layout(_bf16(wk_t[:, el]), 128),
            'wv': _sbuf_layout(_bf16(wv_t[:, el]), 512),
            'wo': _sbuf_layout(_bf16(wo_cat), 512),
            'bq': np.ascontiguousarray(
                qkv_b[r * EL:(r + 1) * EL].reshape(HL, 128)).astype(
                    np.float32),
            'bo': bo,
            'ones': ones,
            'msel': msel,
        })
    return ins


class SpmdRunner:
    """SPMD runner over axon PJRT keeping a reusable jitted callable."""

    def __init__(self, nc, n_cores=8):
        import jax
        from jax.sharding import Mesh, PartitionSpec
        from jax.experimental.shard_map import shard_map
        from concourse import bass2jax
        bass2jax.install_neuronx_cc_hook()
        self.nc = nc
        self.n_cores = n_cores
        partition_name = (
            nc.partition_id_tensor.name if nc.partition_id_tensor else None)
        in_names, out_names, out_avals, zero_outs = [], [], [], []
        for alloc in nc.m.functions[0].allocations:
            if not isinstance(alloc, mybir.MemoryLocationSet):
                continue
            name = alloc.memorylocations[0].name
            if alloc.kind == 'ExternalInput':
                if name != partition_name:
                    in_names.append(name)
            elif alloc.kind == 'ExternalOutput':
                shape = tuple(alloc.tensor_shape)
                dtype = mybir.dt.np(alloc.dtype)
                out_names.append(name)
                out_avals.append(jax.core.ShapedArray(shape, dtype))
                zero_outs.append(np.zeros(shape, dtype))
        self.in_names = in_names
        self.out_names = out_names
        self.out_avals = out_avals
        self.zero_outs = zero_outs
        self.n_params = len(in_names)
        n_outs = len(out_avals)
        all_in_names = list(in_names) + list(out_names)
        if partition_name is not None:
            all_in_names.append(partition_name)

        def _body(*args):
            operands = list(args)
            if partition_name is not None:
                operands.append(bass2jax.partition_id_tensor())
            outs = bass2jax._bass_exec_p.bind(
                *operands,
                out_avals=tuple(out_avals),
                in_names=tuple(all_in_names),
                out_names=tuple(out_names),
                lowering_input_output_aliases=(),
                sim_require_finite=True,
                sim_require_nnan=True,
                nc=nc,
            )
            return tuple(outs)

        import os
        if os.environ.get('BASS_SIM'):
            devices = jax.devices('cpu')[:n_cores]
        else:
            devices = jax.devices()[:n_cores]
        assert len(devices) == n_cores
        self.mesh = Mesh(np.asarray(devices), ('core',))
        in_specs = (PartitionSpec('core'),) * (self.n_params + n_outs)
        out_specs = (PartitionSpec('core'),) * n_outs
        self.fn = jax.jit(
            shard_map(_body, mesh=self.mesh, in_specs=in_specs,
                      out_specs=out_specs, check_rep=False),
            keep_unused=True)
        self._jax = jax

    def pack(self, in_maps):
        per_core = [[np.asarray(m[n]) for n in self.in_names] for m in in_maps]
        concat_in = [
            np.concatenate([per_core[c][i] for c in range(self.n_cores)],
                           axis=0)
            for i in range(self.n_params)]
        concat_zeros = [
            np.zeros((self.n_cores * z.shape[0], *z.shape[1:]), z.dtype)
            for z in self.zero_outs]
        return concat_in + concat_zeros

    def device_put(self, args):
        from jax.sharding import NamedSharding, PartitionSpec
        sh = NamedSharding(self.mesh, PartitionSpec('core'))
        return [self._jax.device_put(a, sh) for a in args]

    def unpack(self, out_arrs):
        return [
            {n: np.asarray(out_arrs[i]).reshape(
                self.n_cores, *self.out_avals[i].shape)[c]
             for i, n in enumerate(self.out_names)}
            for c in range(self.n_cores)]

    def run(self, in_maps):
        return self.unpack(self.fn(*self.pack(in_maps)))

    def time_exec(self, in_maps, iters=20, warmup=3):
        import time as _time
        args = self.device_put(self.pack(in_maps))
        out = None
        for _ in range(warmup):
            out = self.fn(*args)
        self._jax.block_until_ready(out)
        t0 = _time.perf_counter()
        outs = [self.fn(*args) for _ in range(iters)]
        self._jax.block_until_ready(outs)
        return (_time.perf_counter() - t0) / iters


_CACHE = {}


def _get_runner(T=2048, reps=1):
    key = (T, reps)
    if key not in _CACHE:
        nc = build_nc(T=T, reps=reps)
        _CACHE[key] = SpmdRunner(nc, 8)
    return _CACHE[key]


def kernel(x, qkv_w, qkv_b, out_w, out_b):
    B, T, _ = x.shape
    TH = T // 2
    runner = _get_runner(T=T)
    ins = make_inputs(x, qkv_w, qkv_b, out_w, out_b)
    res = runner.run(ins)
    out = np.empty((B, T, D), np.float32)
    for c in range(8):
        b, r = c // 2, c % 2
        out[b, r * TH:(r + 1) * TH, :] = res[c]['y']
    return out
